# revision 16
# baseline (speedup 1.0000x reference)
"""Trainium2 Bass kernel for nn_CoAttention.

Sharding: data-parallel over batch. B=16 across 8 cores -> 2 batches/core.
All weights replicated. No collectives.

Precision: fp16 operands (values are O(1..60)); raw exp(l) tensors (eT/eN)
in bf16 (values up to ~e^70); matmuls accumulate fp32 in PSUM; LSTM cell
state fp32; h and outputs fp16.

Per-core program (per local batch b):
  h_sT = tanh(W_ref @ src_b.T + b_ref)       [H, S]   MM(5th k-tile = bias) + Tanh
  h_sN = tanh(src_b @ W_ref.T + b_ref)       [S, H]   recomputed, no transposes
  eT   = exp(h_r @ h_s.T)                    [R, S]   MM + Exp (Ds accum fused)
  eN   = exp(h_s @ h_r.T)                    [S, R]   recomputed (Dr accum fused)
  hrNs = h_rN / Ds  (per-partition scale)    folds softmax-over-s into c_s
  eS   = eN / Dr                             softmax over r
  c_sT = eT.T @ hrNs                         [S, H]
  c_rT = [h_sN | c_sT].T @ eS                [2H, R]
  xg_d = W_ihT_d.T @ [c_rT; h_rT] + b        [G, R] -> strided scan layout
  LSTM scan fwd+bwd interleaved: gates on partitions, [128,16] PSUM/step,
  xg preloaded into PSUM (DVE copy), matmuls accumulate onto it.
  Sigma-trick: g-gate rows pre-scaled x2 host-side; one Sigmoid ACT over all
  16 cols; tanh(g) = 2*sig(2g)-1 via one fused tensor_scalar.
  h written fp16 directly into the output image; next step's matmul reads it.

Outputs return as raw [128, 4R] fp16 SBUF images, decoded on host.
"""

import numpy as np
import ml_dtypes

import concourse.bass as bass
import concourse.mybir as mybir
import concourse.tile as tile
from concourse import bacc
from concourse import bass_utils

BF16 = ml_dtypes.bfloat16
FP16 = np.float16

B, S, R, H = 16, 1024, 512, 512
HD = H // 2          # 256
G = 4 * HD           # 1024
DIN = 3 * H          # 1536
N_CORES = 8
BLOC = B // N_CORES  # 2
SCAN_STEPS = R       # full scan; lower only for dev experiments
# Gate-type permutation (host side): blocks reordered i,f,o,g.
_GPERM = np.r_[0:512, 768:1024, 512:768]

F32 = mybir.dt.float32
F16 = mybir.dt.float16
BF = mybir.dt.bfloat16
AF = mybir.ActivationFunctionType
ALU = mybir.AluOpType

_CACHE = {}

# scan experiment knobs (module-level so sim scripts can flip them)
PRELOAD_MODE = "dve"   # "dve" | "dma"
GPSIMD_CF = False      # run c*sig(f) on GPSIMD instead of DVE
PSC_BUFS = 2


def _build_nc(scan_steps=None, repeat=1):
    scan_steps = SCAN_STEPS if scan_steps is None else scan_steps
    nc = bacc.Bacc("TRN2", target_bir_lowering=False, debug=False,
                   num_devices=N_CORES)

    # ---- DRAM I/O (all host-prepped [128, F] SBUF images) ----
    # srcT has 5 k-tiles: 4 data + 1 bias tile (row0 = ones).
    # wrefT has 5 k-tiles: 4 data + 1 bias tile (row0 = b_ref).
    d_srcT = nc.dram_tensor("srcT", [128, BLOC * 5 * S], F16, kind="ExternalInput")
    d_hrT = nc.dram_tensor("hrT", [128, BLOC * 4 * R], F16, kind="ExternalInput")
    d_hrN = nc.dram_tensor("hrN", [128, BLOC * 4 * H], BF, kind="ExternalInput")
    d_wrefT = nc.dram_tensor("wrefT", [128, 5 * H], F16, kind="ExternalInput")
    d_wih = {d: nc.dram_tensor(f"wihT_{d}", [128, 12 * G], F16, kind="ExternalInput")
             for d in "fb"}
    d_whh = {d: nc.dram_tensor(f"whhT_{d}", [128, 2 * G], F16, kind="ExternalInput")
             for d in "fb"}
    d_bg = {d: nc.dram_tensor(f"bgT_{d}", [128, 8], F32, kind="ExternalInput")
            for d in "fb"}
    d_id16 = nc.dram_tensor("id16", [128, 128], F16, kind="ExternalInput")
    d_out = {d: nc.dram_tensor(f"out_{d}", [128, 4 * R], F16, kind="ExternalOutput")
             for d in "fb"}

    with tile.TileContext(nc) as tc, \
         tc.tile_pool(name="wp", bufs=1) as wp, \
         tc.tile_pool(name="ap", bufs=1) as ap, \
         tc.tile_pool(name="scansb", bufs=8) as scansb, \
         tc.tile_pool(name="pp", bufs=2, space="PSUM") as pp, \
         tc.tile_pool(name="psc", bufs=PSC_BUFS, space="PSUM") as psc:

        prev_outb = None
        for _rep in range(repeat):
            # ---- persistent loads ----
            def load(dram, shape, dt):
                t = wp.tile(shape, dt, tag=dram.name, name=dram.name)
                nc.sync.dma_start(t[:], dram[:])
                return t

            # load order = first-use order: wrefT/srcT feed the first matmuls;
            # wih/whh/id16 aren't read until the xg (~100us) / scan (~200us)
            # stages, so they queue last and hide behind compute.
            wrefT = load(d_wrefT, [128, 5 * H], F16)
            srcT_pre = ap.tile([128, 5 * S], F16, tag="tagA", name="srcT_pre")
            if prev_outb is not None:
                # serialize reps for honest repeat-slope timing: the dummy
                # copy reads the previous rep's final outputs and writes the
                # head of srcT_pre, which the DMA below then overwrites.
                for _d in "fb":
                    nc.vector.tensor_copy(srcT_pre[:, 0:4], prev_outb[_d][:, 0:4])
            nc.sync.dma_start(srcT_pre[:], d_srcT[:, 0:5 * S])
            hrT = load(d_hrT, [128, BLOC * 4 * R], F16)
            hrN = load(d_hrN, [128, BLOC * 4 * H], BF)
            bg = {d: load(d_bg[d], [128, 8], F32) for d in "fb"}
            wih = {d: load(d_wih[d], [128, 12 * G], F16) for d in "fb"}
            whh = {d: load(d_whh[d], [128, 2 * G], F16) for d in "fb"}
            id16 = load(d_id16, [128, 128], F16)

            xg = {d: wp.tile([128, 16 * R], F16, tag=f"xg_{d}", name=f"xg_{d}")
                  for d in "fb"}
            outb = {d: wp.tile([128, 4 * R], F16, tag=f"outsb_{d}",
                               name=f"outsb_{d}") for d in "fb"}

            # ---- attention + xg, per local batch ----
            for b in range(BLOC):
                hrT_b = hrT[:, b * 4 * R:(b + 1) * 4 * R]
                hrN_b = hrN[:, b * 4 * H:(b + 1) * 4 * H]

                if b == 0:
                    srcT_b = srcT_pre
                else:
                    srcT_b = ap.tile([128, 5 * S], F16, tag="tagA")
                    nc.sync.dma_start(srcT_b[:], d_srcT[:, b * 5 * S:(b + 1) * 5 * S])

                # 1) h_sT [4 Hout-tiles x S] (bias via 5th k-tile)
                hsT = ap.tile([128, 4 * S], F16, tag="tagB")
                for m in range(4):
                    for sc in range(2):
                        ps = pp.tile([128, 512], F32, tag="mm")
                        for k in range(5):
                            nc.tensor.matmul(
                                ps[:],
                                wrefT[:, k * H + m * 128: k * H + (m + 1) * 128],
                                srcT_b[:, k * S + sc * 512: k * S + sc * 512 + 512],
                                start=(k == 0), stop=(k == 4))
                        nc.scalar.activation(
                            hsT[:, m * S + sc * 512: m * S + sc * 512 + 512],
                            ps[:], AF.Tanh)

                # 2) h_sN [8 S-tiles x H] recomputed (bias via 5th k-tile)
                hsN = ap.tile([128, 8 * H], F16, tag="tagC")
                for st in range(8):
                    ps = pp.tile([128, 512], F32, tag="mm")
                    for k in range(5):
                        nc.tensor.matmul(
                            ps[:],
                            srcT_b[:, k * S + st * 128: k * S + st * 128 + 128],
                            wrefT[:, k * H: (k + 1) * H],
                            start=(k == 0), stop=(k == 4))
                    nc.scalar.activation(hsN[:, st * H:(st + 1) * H], ps[:], AF.Tanh)

                # 3) eT [4 R-tiles x S] = exp(l.T), Ds partials into accum_out
                eT = ap.tile([128, 4 * S], BF, tag="tagD")
                ds2 = ap.tile([128, 8], F32, tag="ds2")
                for rt in range(4):
                    for sc in range(2):
                        ps = pp.tile([128, 512], F32, tag="mm")
                        for k in range(4):
                            nc.tensor.matmul(
                                ps[:],
                                hrT_b[:, k * R + rt * 128: k * R + (rt + 1) * 128],
                                hsT[:, k * S + sc * 512: k * S + sc * 512 + 512],
                                start=(k == 0), stop=(k == 3))
                        nc.scalar.activation(
                            eT[:, rt * S + sc * 512: rt * S + sc * 512 + 512],
                            ps[:], AF.Exp,
                            accum_out=ds2[:, rt * 2 + sc: rt * 2 + sc + 1])

                # 4) eN [8 S-tiles x R] recomputed, Dr fused into accum_out
                eN = ap.tile([128, 8 * R], BF, tag="tagE")
                drsum = ap.tile([128, 8], F32, tag="drsum")
                for st in range(8):
                    ps = pp.tile([128, 512], F32, tag="mm")
                    for k in range(4):
                        nc.tensor.matmul(
                            ps[:],
                            hsT[:, k * S + st * 128: k * S + st * 128 + 128],
                            hrT_b[:, k * R: (k + 1) * R],
                            start=(k == 0), stop=(k == 3))
                    nc.scalar.activation(
                        eN[:, st * R:(st + 1) * R], ps[:], AF.Exp,
                        accum_out=drsum[:, st:st + 1])

                # 5) softmax denominators
                dsum = ap.tile([128, 4], F32, tag="dsum")
                for rt in range(4):
                    nc.vector.tensor_add(dsum[:, rt:rt + 1], ds2[:, 2 * rt:2 * rt + 1],
                                         ds2[:, 2 * rt + 1:2 * rt + 2])
                invDs = ap.tile([128, 4], F32, tag="invDs")
                nc.vector.reciprocal(invDs[:], dsum[:])
                invDr = ap.tile([128, 8], F32, tag="invDr")
                nc.vector.reciprocal(invDr[:], drsum[:])

                # 6) hrNs = hrN / Ds (folds softmax-over-s scale into c_s)
                hrNs = ap.tile([128, 4 * H], BF, tag="hrNs")
                for k in range(4):
                    nc.vector.tensor_scalar_mul(
                        hrNs[:, k * H:(k + 1) * H], hrN_b[:, k * H:(k + 1) * H],
                        invDs[:, k:k + 1])
                # 7) eS = eN / Dr
                eS = ap.tile([128, 8 * R], F16, tag="tagG")
                for st in range(8):
                    nc.vector.tensor_scalar_mul(
                        eS[:, st * R:(st + 1) * R], eN[:, st * R:(st + 1) * R],
                        invDr[:, st:st + 1])

                # 8) c_sT [8 S-tiles x H] (reuses srcT slot)
                csT = ap.tile([128, 8 * H], F16, tag="tagA")
                for st in range(8):
                    ps = pp.tile([128, 512], F32, tag="mm")
                    for k in range(4):
                        nc.tensor.matmul(
                            ps[:],
                            eT[:, k * S + st * 128: k * S + st * 128 + 128],
                            hrNs[:, k * H: (k + 1) * H],
                            start=(k == 0), stop=(k == 3))
                    nc.any.tensor_copy(csT[:, st * H:(st + 1) * H], ps[:])

                # 9) c_rT [8 2H-tiles x R] (reuses hsT slot)
                crT = ap.tile([128, 8 * R], F16, tag="tagB2")
                for m in range(8):
                    ps = pp.tile([128, 512], F32, tag="mm")
                    for k in range(8):
                        if m < 4:
                            lhsT = hsN[:, k * H + m * 128: k * H + m * 128 + 128]
                        else:
                            lhsT = csT[:, k * H + (m - 4) * 128: k * H + (m - 4) * 128 + 128]
                        nc.tensor.matmul(ps[:], lhsT, eS[:, k * R:(k + 1) * R],
                                         start=(k == 0), stop=(k == 7))
                    nc.any.tensor_copy(crT[:, m * R:(m + 1) * R], ps[:])

                # 10) xg per direction, strided into scan layout (col = 16t+2g+b)
                for d in "fb":
                    for g in range(8):
                        ps = pp.tile([128, 512], F32, tag="mm")
                        for k in range(12):
                            if k < 8:
                                rhs = crT[:, k * R:(k + 1) * R]
                            else:
                                rhs = hrT_b[:, (k - 8) * R:(k - 7) * R]
                            nc.tensor.matmul(
                                ps[:],
                                wih[d][:, k * G + g * 128: k * G + (g + 1) * 128],
                                rhs, start=(k == 0), stop=(k == 11))
                        dst = xg[d][:, 2 * g + b:: 16]
                        nc.vector.tensor_scalar_add(dst, ps[:], bg[d][:, g:g + 1])

            # ---- LSTM scan ----
            # per-step PSUM [128,16]: col = 2*gtile + batch; gate blocks
            # (host GPERM) i 0:4, f 4:8, o 8:12, g 12:16 (g pre-scaled x2).
            h0 = wp.tile([128, 4], F16, tag="h0", name="h0")
            cst = {d: wp.tile([128, 4], F32, tag=f"c_{d}", name=f"c_{d}") for d in "fb"}
            nc.vector.memset(h0[:], 0.0)
            for d in "fb":
                nc.vector.memset(cst[d][:], 0.0)
                if scan_steps < R:
                    nc.vector.memset(outb[d][:], 0.0)

            # xg preloaded into PSUM in 32-step blocks: one [128,512] copy per
            # block per dir; each step's matmuls accumulate onto its 16-col
            # slice. For dir b the in-block position runs backwards.
            NBLK = 32
            psb = {}
            for t in range(scan_steps):
                for d in "fb":
                    te = t if d == "f" else R - 1 - t
                    tp = te - 1 if d == "f" else te + 1
                    j = t % NBLK
                    if j == 0:
                        lo = t if d == "f" else R - NBLK - t
                        psb[d] = psc.tile([128, 16 * NBLK], F32, tag=f"scps_{d}", name=f"scps_{d}")
                        nc.tensor.matmul(
                            psb[d][:], id16[:],
                            xg[d][:, 16 * lo:16 * (lo + NBLK)],
                            start=True, stop=True)
                    pos = j if d == "f" else NBLK - 1 - j
                    ps = psb[d][:, 16 * pos:16 * pos + 16]
                    for g in range(8):
                        for k in range(2):
                            rhs = (h0[:, 2 * k:2 * k + 2] if t == 0 else
                                   outb[d][:, 4 * tp + 2 * k: 4 * tp + 2 * k + 2])
                            nc.tensor.matmul(
                                ps[:, 2 * g:2 * g + 2],
                                whh[d][:, k * G + g * 128: k * G + (g + 1) * 128],
                                rhs,
                                start=False, stop=(k == 1),
                                skip_group_check=True)
                    acts = scansb.tile([128, 16], F32, tag="acts")
                    nc.scalar.activation(acts[:], ps[:], AF.Sigmoid)
                    # tanh(g) = 2*sig(2g) - 1
                    tg = scansb.tile([128, 4], F32, tag="tg")
                    nc.vector.tensor_scalar(tg[:], acts[:, 12:16], 2.0, -1.0,
                                            ALU.mult, ALU.add)
                    t1 = scansb.tile([128, 4], F32, tag="t1")
                    nc.vector.tensor_mul(t1[:], acts[:, 0:4], tg[:])
                    if GPSIMD_CF:
                        nc.gpsimd.tensor_mul(cst[d][:], cst[d][:], acts[:, 4:8])
                    else:
                        nc.vector.tensor_mul(cst[d][:], cst[d][:], acts[:, 4:8])
                    nc.vector.tensor_add(cst[d][:], cst[d][:], t1[:])
                    tc2 = scansb.tile([128, 4], F32, tag="tc2")
                    nc.scalar.activation(tc2[:], cst[d][:], AF.Tanh)
                    nc.vector.tensor_mul(outb[d][:, 4 * te:4 * te + 4],
                                         tc2[:], acts[:, 8:12])

            for d in "fb":
                nc.sync.dma_start(d_out[d][:], outb[d][:])
            prev_outb = outb

    nc.compile()
    return nc


def _build_noop_nc():
    """Same DRAM I/O footprint as _build_nc, near-zero device work.

    Used by timing.py to measure the fixed per-call RPC/dispatch floor of
    the axon tunnel; (kernel wall - noop wall) isolates device exec time.
    """
    nc = bacc.Bacc("TRN2", target_bir_lowering=False, debug=False,
                   num_devices=N_CORES)
    nc.dram_tensor("srcT", [128, BLOC * 5 * S], F16, kind="ExternalInput")
    nc.dram_tensor("hrT", [128, BLOC * 4 * R], F16, kind="ExternalInput")
    nc.dram_tensor("hrN", [128, BLOC * 4 * H], BF, kind="ExternalInput")
    d_wrefT = nc.dram_tensor("wrefT", [128, 5 * H], F16, kind="ExternalInput")
    for d in "fb":
        nc.dram_tensor(f"wihT_{d}", [128, 12 * G], F16, kind="ExternalInput")
        nc.dram_tensor(f"whhT_{d}", [128, 2 * G], F16, kind="ExternalInput")
        nc.dram_tensor(f"bgT_{d}", [128, 8], F32, kind="ExternalInput")
    nc.dram_tensor("id16", [128, 128], F16, kind="ExternalInput")
    d_out = {d: nc.dram_tensor(f"out_{d}", [128, 4 * R], F16,
                               kind="ExternalOutput") for d in "fb"}
    with tile.TileContext(nc) as tc, tc.tile_pool(name="np0", bufs=1) as pool:
        t = pool.tile([128, 4], F16, tag="t")
        nc.sync.dma_start(t[:], d_wrefT[:, 0:4])
        for d in "fb":
            nc.sync.dma_start(d_out[d][:, 0:4], t[:])
    nc.compile()
    return nc


def _img_kmaj(x, p=128):
    """[K, F] -> [128, (K/128)*F] k-tile image."""
    k, f = x.shape
    return np.ascontiguousarray(
        x.reshape(k // p, p, f).transpose(1, 0, 2).reshape(p, (k // p) * f))


def _prep_core(core, inp):
    gb = [BLOC * core + i for i in range(BLOC)]
    src = np.asarray(inp["src_memory_bank"])   # [S, B, H]
    ref = np.asarray(inp["ref_memory_bank"])   # [R, B, H]
    bref = np.asarray(inp["b_ref"]).astype(np.float32)

    def cat(imgs):
        return np.concatenate(imgs, axis=1)

    def src_img(b):
        x = np.zeros((5 * 128, S), dtype=FP16)
        x[:H] = src[:, b, :].T.astype(FP16)
        x[H] = 1.0
        return _img_kmaj(x)

    m = {}
    m["srcT"] = cat([src_img(b) for b in gb])
    m["hrT"] = cat([_img_kmaj(ref[:, b, :].T.astype(FP16)) for b in gb])
    m["hrN"] = cat([_img_kmaj(ref[:, b, :].astype(BF16)) for b in gb])
    wr = np.zeros((5 * 128, H), dtype=FP16)
    wr[:H] = np.asarray(inp["W_ref"]).T.astype(FP16)
    wr[H] = bref.astype(FP16)
    m["wrefT"] = _img_kmaj(wr)
    for d, sfx in (("f", "_f"), ("b", "_b")):
        wih = np.asarray(inp[f"W_ih{sfx}"])[_GPERM].astype(np.float32)
        whh = np.asarray(inp[f"W_hh{sfx}"])[_GPERM].astype(np.float32)
        bsum = (np.asarray(inp[f"b_ih{sfx}"], dtype=np.float64)
                + np.asarray(inp[f"b_hh{sfx}"], dtype=np.float64))[_GPERM]
        # sigma-trick: pre-scale g-gate block (rows 768:1024) by 2
        wih[768:1024] *= 2.0
        whh[768:1024] *= 2.0
        bsum = bsum.astype(np.float32)
        bsum[768:1024] *= 2.0
        m[f"wihT_{d}"] = _img_kmaj(wih.T.astype(FP16))
        m[f"whhT_{d}"] = _img_kmaj(whh.T.astype(FP16))
        m[f"bgT_{d}"] = np.ascontiguousarray(bsum.reshape(8, 128).T)
    m["id16"] = np.eye(128, dtype=FP16)
    return m


def _decode(res_list):
    """results -> [R, B, H] fp32"""
    out = np.zeros((R, B, H), dtype=np.float32)
    for c in range(N_CORES):
        for d, off in (("f", 0), ("b", HD)):
            img = np.asarray(res_list[c][f"out_{d}"]).astype(np.float32)
            x = img.reshape(128, R, 2, BLOC).transpose(1, 3, 2, 0)  # t,b,k,p
            x = np.ascontiguousarray(x).reshape(R, BLOC, HD)
            out[:, BLOC * c:BLOC * (c + 1), off:off + HD] = x
    return out


def kernel(**inputs):
    if "nc" not in _CACHE:
        _CACHE["nc"] = _build_nc()
    nc = _CACHE["nc"]
    in_maps = [_prep_core(c, inputs) for c in range(N_CORES)]
    res = bass_utils.run_bass_kernel_spmd(nc, in_maps,
                                          core_ids=list(range(N_CORES)))
    return _decode(res.results)
